# revision 19
# baseline (speedup 1.0000x reference)
"""Trainium2 Bass kernel for a 4-layer decoder (nn_Decoder_46531675685089).

Sharding: Megatron TP-8 (2 heads/core for attention, F/8=512 FFN cols/core).
Activations flow feature-major [d, tok] through matmuls with tok order
(rank, batch, s_local); out-proj/FFN2 use the activation tile as the
stationary operand so partial sums emerge token-major, ready for
ReduceScatter.  Each core then BatchNorms only its own token chunk
(BN stats per-s over (b, d) are fully local after RS), transposes the
small chunk back to feature-major and AllGathers.

Pipelining: every stage boundary is split into two halves along s_local
(A = s_local 0..63, B = 64..127).  Attention/FFN compute for half B and
the BN of half A overlap the (serialized) collective stream
RS_A, RS_B, AG_A, AG_B; the next stage's q/k/v (or FFN1) for half A
starts as soon as AG_A lands, overlapping AG_B.

Matmuls and exchange buffers are bf16 (fp32 PSUM accumulation);
BN/residual math is fp32.  MHA1 computes only the unmasked first 512
key positions.  Softmax denominators come from a ones-row appended to
V; normalization uses a K=1 broadcast matmul of the reciprocal row.
"""

import numpy as np
import ml_dtypes

import concourse.bass as bass
import concourse.mybir as mybir
import concourse.tile as tile
from concourse import bacc
from concourse.bass_utils import run_bass_kernel_spmd
from concourse.masks import make_identity

F32 = mybir.dt.float32
BF16 = mybir.dt.bfloat16
NPBF16 = ml_dtypes.bfloat16

R = 8            # cores
L = 4            # layers
B = 2            # batch
S = 1024         # sequence
D = 1024         # model dim
HLOC = 2         # heads per core
DK = 64
FLOC = 512       # FFN cols per core
CH = 128         # s positions per core
SH = 64          # s positions per half-chunk
NT = B * S       # 2048 tokens, order (r, b, s_local)
NTH = NT // 2    # tokens per s-half: (r, b, s64) = 1024
MASK = 512       # key positions >= MASK are masked in MHA1
EPS = 1e-5

AluOp = mybir.AluOpType
Act = mybir.ActivationFunctionType


# ---------------------------------------------------------------- builder --

def build_kernel(nc):
    # ---------------- DRAM I/O ----------------
    t_in = {}
    def ein(name, shape, dt):
        t_in[name] = nc.dram_tensor(name, list(shape), dt, kind="ExternalInput")
        return t_in[name]

    x_chunk = ein("x_chunk", (B, CH, D), F32)
    xT0 = ein("xT0", (R, D, B, CH), BF16)
    wq = {i: ein(f"wq{i}", (L, 8, 128, 128), BF16) for i in (1, 2)}
    wk = {i: ein(f"wk{i}", (L, 8, 128, 128), BF16) for i in (1, 2)}
    wv = {i: ein(f"wv{i}", (L, 8, 128, 128), BF16) for i in (1, 2)}
    wo = {i: ein(f"wo{i}", (L, 128, D), BF16) for i in (1, 2)}
    bqkv = {i: ein(f"bqkv{i}", (L, 3, 128), F32) for i in (1, 2)}
    w1 = ein("w1", (L, 8, 128, FLOC), BF16)
    w2 = ein("w2", (L, 4, 128, D), BF16)
    bf1 = ein("bf1", (L, 4, 128), F32)
    bias_bc = ein("bias_bc", (L, 3, D), F32)     # bo1, bo2, bf2 rows
    gbe = ein("gbe", (L, 3, 2, CH), F32)         # per-core g/be slices

    out_chunk = nc.dram_tensor("out_chunk", [B, CH, D], F32, kind="ExternalOutput")

    # ---------------- internal DRAM (exchange), per parity x s-half ----------
    rs_in = [[nc.dram_tensor(f"rs_in{p}{h}", [NTH, D], BF16) for h in range(2)]
             for p in range(2)]
    rs_out = [[nc.dram_tensor(f"rs_out{p}{h}", [B * SH, D], BF16) for h in range(2)]
              for p in range(2)]
    ag_in = [[nc.dram_tensor(f"ag_in{p}{h}", [D, B, SH], BF16) for h in range(2)]
             for p in range(2)]
    ag_out = [[nc.dram_tensor(f"ag_out{p}{h}", [R, D, B, SH], BF16,
                              addr_space="Shared") for h in range(2)]
              for p in range(2)]

    groups = [list(range(R))]

    with tile.TileContext(nc) as tc:
        import contextlib
        ctx = contextlib.ExitStack()
        with ctx:
            consts = ctx.enter_context(tc.tile_pool(name="consts", bufs=1))
            wpool = ctx.enter_context(tc.tile_pool(name="weights", bufs=1))
            xpool = ctx.enter_context(tc.tile_pool(name="xT", bufs=1))
            qkv_pool = ctx.enter_context(tc.tile_pool(name="qkv", bufs=1))
            attn_pool = ctx.enter_context(tc.tile_pool(name="attn", bufs=2))
            res_pool = ctx.enter_context(tc.tile_pool(name="res", bufs=2))
            chunk_pool = ctx.enter_context(tc.tile_pool(name="chunk", bufs=1))
            stat_pool = ctx.enter_context(tc.tile_pool(name="stats", bufs=4))
            rd_pool = ctx.enter_context(tc.tile_pool(name="rd", bufs=1))
            out_pool = ctx.enter_context(tc.tile_pool(name="outp", bufs=2))
            ps_mm = ctx.enter_context(tc.tile_pool(name="ps_mm", bufs=6, space="PSUM"))
            ps_sm = ctx.enter_context(tc.tile_pool(name="ps_sm", bufs=2, space="PSUM"))

            # constants
            ident32 = consts.tile([128, 128], F32)
            make_identity(nc, ident32)
            ident16 = consts.tile([128, 128], BF16)
            make_identity(nc, ident16)
            ones16 = consts.tile([1, 64], BF16)
            nc.vector.memset(ones16, 1.0)
            eps_t = consts.tile([128, 1], F32)
            nc.vector.memset(eps_t, EPS)

            # ---------------- helpers ----------------
            def load_xT_half(xT, src4, hf):
                """src4: DRAM AP [R, D, B, SH] for half hf -> xT slice."""
                x5 = xT.rearrange("p jd (r b s) -> p jd r b s", b=B, s=CH)
                s4 = src4.rearrange("r (jd dp) b s -> dp jd r b s", dp=128)
                for jd in range(8):
                    for b in range(B):
                        nc.sync.dma_start(
                            out=x5[:, jd, :, b, SH * hf:SH * hf + SH],
                            in_=s4[:, jd, :, b])

            def load_xT_full(src4):
                xT = xpool.tile([128, 8, NT], BF16, tag="xT")
                s4 = src4.rearrange("r (jd dp) b s -> dp jd r b s", dp=128)
                for jd in range(8):
                    nc.sync.dma_start(
                        out=xT[:, jd].rearrange("p (r b s) -> p r b s", b=B, s=CH),
                        in_=s4[:, jd])
                return xT

            def proj_half(xT, w_sb, b_sb, hf, grange, o_sb):
                """q/k/v projection for s-half hf: writes o_sb cols
                (rb in grange*8.., s in half) with bias.  o_sb [128,16,128]."""
                x5 = xT.rearrange("p jd (rb s) -> p jd rb s", s=CH)
                pss = []
                for g in grange:
                    ps = ps_mm.tile([128, 512], F32, tag="mm", name=f"qkv{hf}{g}")
                    psv = ps.rearrange("p (rb s) -> p rb s", s=SH)
                    for jd in range(8):
                        nc.tensor.matmul(
                            psv, w_sb[:, jd],
                            x5[:, jd, 8 * g:8 * g + 8, SH * hf:SH * hf + SH],
                            start=(jd == 0), stop=(jd == 7))
                    pss.append((g, psv))
                for g, psv in pss:
                    nc.vector.tensor_scalar_add(
                        o_sb[:, 8 * g:8 * g + 8, SH * hf:SH * hf + SH], psv, b_sb)

            def bn_half(li, si, par, hf, res_h, last):
                """Bias + residual + BN for s-half hf.  Returns xn_h [64,B,D]."""
                bb = chunk_pool.tile([128, D], F32, tag="bb")
                brow = bias_bc.ap()[li, si]      # [D]
                nc.sync.dma_start(
                    out=bb,
                    in_=bass.AP(tensor=brow.tensor, offset=brow.offset,
                                ap=[[0, 128]] + brow.ap))
                g_sb = stat_pool.tile([64, 2], F32, tag="gbe")
                nc.sync.dma_start(
                    out=g_sb,
                    in_=gbe.ap()[li, si, :, SH * hf:SH * hf + SH]
                        .rearrange("n s -> s n"))

                ch = chunk_pool.tile([64, B, D], BF16, tag="ch")
                nc.sync.dma_start(
                    out=ch,
                    in_=rs_out[par][hf].ap().rearrange("(b s) d -> s b d", b=B))
                u = chunk_pool.tile([64, B, D], F32, tag="u")
                stats = stat_pool.tile([64, 2 * B, 6], F32, tag="bnst")
                for b in range(B):
                    nc.vector.tensor_add(u[:, b], ch[:, b], res_h[:, b])
                    nc.vector.tensor_add(u[:, b], u[:, b], bb[0:64])
                    for half in range(2):
                        nc.vector.bn_stats(stats[:, 2 * b + half],
                                           u[:, b, 512 * half:512 * half + 512])
                mv = stat_pool.tile([64, 2], F32, tag="mv")
                nc.vector.bn_aggr(mv, stats)
                std = stat_pool.tile([64, 1], F32, tag="std")
                nc.scalar.activation(std, mv[:, 1:2], Act.Sqrt, bias=eps_t[0:64])
                rstd = stat_pool.tile([64, 1], F32, tag="rstd")
                nc.vector.reciprocal(rstd, std)
                A_t = stat_pool.tile([64, 1], F32, tag="A")
                nc.vector.tensor_mul(A_t, rstd, g_sb[:, 0:1])
                mA = stat_pool.tile([64, 1], F32, tag="mA")
                nc.vector.tensor_mul(mA, mv[:, 0:1], A_t)
                B_t = stat_pool.tile([64, 1], F32, tag="B")
                nc.vector.tensor_sub(B_t, g_sb[:, 1:2], mA)

                keep = si != 1
                if keep:
                    xn = res_pool.tile([64, B, D], F32, tag=f"res{hf}",
                                       name=f"xn{hf}")
                else:
                    xn = u   # normalize in place; y-chunk is not a residual
                for b in range(B):
                    nc.vector.tensor_scalar(xn[:, b], u[:, b], A_t, B_t,
                                            AluOp.mult, AluOp.add)
                return xn

            def ship_half(par, hf, xn, xT):
                """Transpose normalized half chunk, AG it, land into xT."""
                for jd in range(8):
                    tx = out_pool.tile([128, B, SH], BF16, tag="tx")
                    for b in range(B):
                        tp = ps_sm.tile([128, SH], F32, tag="tp")
                        nc.tensor.transpose(
                            tp, xn[:, b, 128 * jd:128 * jd + 128],
                            ident32[0:64, 0:64])
                        nc.vector.tensor_copy(tx[:, b], tp)
                    nc.sync.dma_start(out=ag_in[par][hf].ap()[128 * jd:128 * jd + 128],
                                      in_=tx)
                nc.gpsimd.collective_compute(
                    "AllGather", AluOp.bypass, replica_groups=groups,
                    ins=[ag_in[par][hf].ap()], outs=[ag_out[par][hf].ap()])
                load_xT_half(xT, ag_out[par][hf].ap(), hf)

            def stage(li, si, xT, xT_new, res_h2, masked=None):
                """One Megatron stage.  res_h2 = (resA, resB) residual halves.
                Writes next x into xT_new (or out_chunk on the last stage).
                Returns (xnA, xnB)."""
                par = (li * 3 + si) % 2
                last = (li == L - 1 and si == 2)

                if si in (0, 1):
                    i = si + 1
                    KVG = 1 if si == 0 else 2     # kv rb-groups per half
                    KVR = 4 * KVG                 # kv r-tiles per batch
                    # weights
                    wq_sb = wpool.tile([128, 8, 128], BF16, tag="wq")
                    wk_sb = wpool.tile([128, 8, 128], BF16, tag="wk")
                    wv_sb = wpool.tile([128, 8, 128], BF16, tag="wv")
                    wo_sb = wpool.tile([64, HLOC, D], BF16, tag="wo")
                    bq_sb = stat_pool.tile([128, 3], F32, tag="bqkv")
                    nc.sync.dma_start(out=wq_sb, in_=wq[i].ap()[li].rearrange("jd dp k -> dp jd k"))
                    nc.sync.dma_start(out=wk_sb, in_=wk[i].ap()[li].rearrange("jd dp k -> dp jd k"))
                    nc.sync.dma_start(out=wv_sb, in_=wv[i].ap()[li].rearrange("jd dp k -> dp jd k"))
                    nc.sync.dma_start(out=wo_sb,
                                      in_=wo[i].ap()[li].rearrange("(h p) d -> p h d", p=64))
                    nc.sync.dma_start(out=bq_sb, in_=bqkv[i].ap()[li].rearrange("n p -> p n"))

                    qT = qkv_pool.tile([128, 16, CH], BF16, tag="qT", name="qT")
                    kT = qkv_pool.tile([128, 16, CH], BF16, tag="kT", name="kT")
                    vT = qkv_pool.tile([128, 16, CH], BF16, tag="vT", name="vT")
                    for hf in range(2):
                        proj_half(xT, wq_sb, bq_sb[:, 0:1], hf, range(2), qT)
                        proj_half(xT, wk_sb, bq_sb[:, 1:2], hf, range(KVG), kT)
                        proj_half(xT, wv_sb, bq_sb[:, 2:3], hf, range(KVG), vT)

                    # per-head attention outputs, token order (hf, r, b, s64)
                    aT_h = [qkv_pool.tile([64, 2, NTH], BF16, tag=f"aT{h}",
                                          name=f"aT{h}")
                            for h in range(HLOC)]
                    q4 = qT.rearrange("p (r b) s -> p r b s", b=B)
                    k4 = kT.rearrange("p (r b) s -> p r b s", b=B)
                    v4 = vT.rearrange("p (r b) s -> p r b s", b=B)

                    # v tok-major: per head 65 cols [v(64) | ones] -> den row
                    vaug_b = []
                    for b in range(B):
                        vaug = qkv_pool.tile([128, KVR, 130], BF16,
                                             tag=f"vaug{b}", name=f"vaug{b}")
                        nc.vector.memset(vaug, 1.0)
                        for kt in range(KVR):
                            tp = ps_sm.tile([128, 128], BF16, tag="tp")
                            nc.tensor.transpose(tp, v4[:, kt, b], ident16)
                            nc.vector.tensor_copy(vaug[:, kt, 0:64], tp[:, 0:64])
                            nc.vector.tensor_copy(vaug[:, kt, 65:129], tp[:, 64:128])
                        vaug_b.append(vaug)

                    for hf in range(2):
                        sh = slice(SH * hf, SH * hf + SH)
                        for b in range(B):
                            vaug = vaug_b[b]
                            expT = [attn_pool.tile([128, KVR, 512], BF16,
                                                   tag=f"expT{h}", name=f"expT{h}")
                                    for h in range(HLOC)]
                            for kt in range(KVR):
                                scs = []
                                for h in range(HLOC):
                                    hp = slice(64 * h, 64 * h + 64)
                                    sc = ps_mm.tile([128, 512], F32, tag="mm",
                                                    name=f"sc{h}")
                                    nc.tensor.matmul(
                                        sc.rearrange("p (r s) -> p r s", s=SH),
                                        k4[hp, kt, b], q4[hp, :, b, sh],
                                        start=True, stop=True)
                                    scs.append(sc)
                                for h in range(HLOC):
                                    nc.scalar.activation(
                                        expT[h][:, kt], scs[h],
                                        Act.Exp, scale=1.0 / np.sqrt(DK))
                            avs = [ps_mm.tile([65, 512], F32, tag="mm",
                                              name=f"avps{h}") for h in range(HLOC)]
                            for h in range(HLOC):
                                for kt in range(KVR):
                                    nc.tensor.matmul(
                                        avs[h], vaug[:, kt, 65 * h:65 * h + 65],
                                        expT[h][:, kt],
                                        start=(kt == 0), stop=(kt == KVR - 1))
                            for h in range(HLOC):
                                av = avs[h]
                                avsb = attn_pool.tile([65, 512], F32, tag="avsb")
                                nc.scalar.copy(avsb, av)
                                rec = rd_pool.tile([1, 512], F32, tag="rec")
                                den = rd_pool.tile([1, 512], F32, tag="den")
                                nc.sync.dma_start(out=den, in_=avsb[64:65, :])
                                nc.vector.reciprocal_approx_fast(rec, den)
                                rec16 = rd_pool.tile([1, 512], BF16, tag="rec16",
                                                     name="rec16")
                                nc.vector.tensor_copy(rec16, rec)
                                bc = ps_sm.tile([64, 512], F32, tag="tp")
                                nc.tensor.matmul(bc, ones16, rec16, start=True,
                                                 stop=True)
                                bcs = attn_pool.tile([64, 512], BF16, tag="bcs")
                                nc.vector.tensor_copy(bcs, bc)
                                ah5 = aT_h[h].rearrange(
                                    "p hf (r b s) -> p hf r b s", b=B, s=SH)
                                nc.vector.tensor_mul(
                                    ah5[:, hf, :, b, :],
                                    avsb[0:64].rearrange("p (r s) -> p r s", s=SH),
                                    bcs.rearrange("p (r s) -> p r s", s=SH))

                        # out-proj for half hf: 8 token tiles of 128
                        for t in range(8):
                            po = out_pool.tile([128, D], BF16, tag="po")
                            pss = [ps_mm.tile([128, 512], F32, tag="mm",
                                              name=f"opps{nh}") for nh in range(2)]
                            for h in range(HLOC):
                                lhs = aT_h[h][:, hf, 128 * t:128 * t + 128]
                                for nh in range(2):
                                    nc.tensor.matmul(
                                        pss[nh], lhs,
                                        wo_sb[:, h, 512 * nh:512 * nh + 512],
                                        start=(h == 0), stop=(h == HLOC - 1))
                            nc.vector.tensor_copy(po[:, 0:512], pss[0])
                            nc.scalar.copy(po[:, 512:1024], pss[1])
                            nc.sync.dma_start(
                                out=rs_in[par][hf].ap()[128 * t:128 * t + 128],
                                in_=po)
                        nc.gpsimd.collective_compute(
                            "ReduceScatter", AluOp.add, replica_groups=groups,
                            ins=[rs_in[par][hf].ap()], outs=[rs_out[par][hf].ap()])
                else:
                    # FFN
                    w1_sb = wpool.tile([128, 8, FLOC], BF16, tag="w1")
                    w2_sb = wpool.tile([128, 4, D], BF16, tag="w2")
                    bf1_sb = stat_pool.tile([128, 4], F32, tag="bf1")
                    nc.sync.dma_start(out=w1_sb, in_=w1.ap()[li].rearrange("jd dp f -> dp jd f"))
                    nc.sync.dma_start(out=w2_sb, in_=w2.ap()[li].rearrange("jf fp d -> fp jf d"))
                    nc.sync.dma_start(out=bf1_sb, in_=bf1.ap()[li].rearrange("jf fp -> fp jf"))

                    x5 = xT.rearrange("p jd (rb s) -> p jd rb s", s=CH)
                    for hf in range(2):
                        sh = slice(SH * hf, SH * hf + SH)
                        hidT = attn_pool.tile([128, 4, NTH], BF16,
                                              tag="expT0", name=f"hidT{hf}")
                        for jf in range(4):
                            pss = []
                            for g in range(2):
                                ps = ps_mm.tile([128, 512], F32, tag="mm",
                                                name=f"f1ps{g}")
                                psv = ps.rearrange("p (rb s) -> p rb s", s=SH)
                                for jd in range(8):
                                    nc.tensor.matmul(
                                        psv, w1_sb[:, jd, 128 * jf:128 * jf + 128],
                                        x5[:, jd, 8 * g:8 * g + 8, sh],
                                        start=(jd == 0), stop=(jd == 7))
                                pss.append((g, psv))
                            for g, psv in pss:
                                nc.scalar.activation(
                                    hidT[:, jf, 512 * g:512 * g + 512],
                                    psv.rearrange("p rb s -> p (rb s)"),
                                    Act.Relu, bias=bf1_sb[:, jf:jf + 1])
                        for t in range(8):
                            po = out_pool.tile([128, D], BF16, tag="po")
                            pss = [ps_mm.tile([128, 512], F32, tag="mm",
                                              name=f"f2ps{nh}") for nh in range(2)]
                            for jf in range(4):
                                lhs = hidT[:, jf, 128 * t:128 * t + 128]
                                for nh in range(2):
                                    nc.tensor.matmul(
                                        pss[nh], lhs,
                                        w2_sb[:, jf, 512 * nh:512 * nh + 512],
                                        start=(jf == 0), stop=(jf == 3))
                            nc.vector.tensor_copy(po[:, 0:512], pss[0])
                            nc.scalar.copy(po[:, 512:1024], pss[1])
                            nc.sync.dma_start(
                                out=rs_in[par][hf].ap()[128 * t:128 * t + 128],
                                in_=po)
                        nc.gpsimd.collective_compute(
                            "ReduceScatter", AluOp.add, replica_groups=groups,
                            ins=[rs_in[par][hf].ap()], outs=[rs_out[par][hf].ap()])

                # ---- per-half: BN then AG ----
                xns = []
                for hf in range(2):
                    xn = bn_half(li, si, par, hf, res_h2[hf], last)
                    if last:
                        oc = out_chunk.ap().rearrange("b s d -> s b d")
                        nc.sync.dma_start(out=oc[SH * hf:SH * hf + SH], in_=xn)
                    else:
                        ship_half(par, hf, xn, xT_new)
                    xns.append(xn)
                return xns

            # ---------------- main program ----------------
            resA = res_pool.tile([64, B, D], F32, tag="res0")
            resB = res_pool.tile([64, B, D], F32, tag="res1")
            xc = x_chunk.ap().rearrange("b s d -> s b d")
            nc.sync.dma_start(out=resA, in_=xc[0:SH])
            nc.sync.dma_start(out=resB, in_=xc[SH:CH])
            xT = load_xT_full(xT0.ap())

            for li in range(L):
                h_res = (resA, resB)
                xT2 = xpool.tile([128, 8, NT], BF16, tag="xT")
                x1 = stage(li, 0, xT, xT2, h_res)
                xT3 = xpool.tile([128, 8, NT], BF16, tag="xT")
                _y = stage(li, 1, xT2, xT3, h_res)
                if li == L - 1:
                    stage(li, 2, xT3, None, x1)
                else:
                    xT4 = xpool.tile([128, 8, NT], BF16, tag="xT")
                    resA, resB = stage(li, 2, xT3, xT4, x1)
                    xT = xT4

    return nc


# ---------------------------------------------------------------- host ----

_CACHE = {}


def _get_compiled():
    if "nc" not in _CACHE:
        nc = bacc.Bacc("TRN2", target_bir_lowering=False, debug=False,
                       num_devices=R)
        build_kernel(nc)
        nc.compile()
        _CACHE["nc"] = nc
    return _CACHE["nc"]


def _prep_core_inputs(inp, c):
    """Per-core input map (numpy)."""
    f32 = np.float32

    def bf(a):
        return np.ascontiguousarray(np.asarray(a, f32).astype(NPBF16))

    x = np.asarray(inp["x"], f32)
    m = {}
    m["x_chunk"] = np.ascontiguousarray(x[:, c * CH:(c + 1) * CH, :])
    m["xT0"] = bf(x.reshape(B, R, CH, D).transpose(1, 3, 0, 2))  # r d b s
    for i in (1, 2):
        for nm, w in (("wq", inp[f"Wq{i}"]), ("wk", inp[f"Wk{i}"]),
                      ("wv", inp[f"Wv{i}"])):
            wc = np.asarray(w, f32)[:, 2 * c:2 * c + 2]          # L 2 D 64
            wc = wc.transpose(0, 2, 1, 3).reshape(L, D, 128)     # L d (h k)
            m[f"{nm}{i}"] = bf(wc.reshape(L, 8, 128, 128))
        m[f"wo{i}"] = bf(np.asarray(inp[f"Wo{i}"], f32)[:, 128 * c:128 * c + 128, :])
        bq = np.asarray(inp[f"bq{i}"], f32)[:, 2 * c:2 * c + 2].reshape(L, 128)
        bk = np.asarray(inp[f"bk{i}"], f32)[:, 2 * c:2 * c + 2].reshape(L, 128)
        bv = np.asarray(inp[f"bv{i}"], f32)[:, 2 * c:2 * c + 2].reshape(L, 128)
        m[f"bqkv{i}"] = np.ascontiguousarray(np.stack([bq, bk, bv], axis=1))
    m["w1"] = bf(np.asarray(inp["W1"], f32)[:, :, FLOC * c:FLOC * (c + 1)]
                 .reshape(L, 8, 128, FLOC))
    m["w2"] = bf(np.asarray(inp["W2"], f32)[:, FLOC * c:FLOC * (c + 1), :]
                 .reshape(L, 4, 128, D))
    m["bf1"] = np.ascontiguousarray(
        np.asarray(inp["bf1"], f32)[:, FLOC * c:FLOC * (c + 1)].reshape(L, 4, 128))
    m["bias_bc"] = np.ascontiguousarray(np.stack(
        [np.asarray(inp["bo1"], f32), np.asarray(inp["bo2"], f32),
         np.asarray(inp["bf2"], f32)], axis=1))
    sl = slice(CH * c, CH * (c + 1))
    m["gbe"] = np.ascontiguousarray(np.stack(
        [np.stack([np.asarray(inp[f"g{j}"], f32)[:, sl],
                   np.asarray(inp[f"be{j}"], f32)[:, sl]], axis=1)
         for j in (1, 2, 3)], axis=1))
    return m


def kernel(**inputs):
    nc = _get_compiled()
    in_maps = [_prep_core_inputs(inputs, c) for c in range(R)]
    res = run_bass_kernel_spmd(nc, in_maps, list(range(R)))
    chunks = [res.results[c]["out_chunk"] for c in range(R)]
    out = np.concatenate(chunks, axis=1).astype(np.float32)
    return out
